# revision 94
# baseline (speedup 1.0000x reference)
"""Multi-head attention Trainium2 kernel (nn_MultiHeadAttention_86423331930281).

Self-contained: builds a Bass/Tile SPMD kernel, data-parallel over batch
(B=8 -> one batch element per NeuronCore), runs on cores 0-7 via
run_bass_kernel_spmd, returns the full [8, 1024, 1024] f32 output.

Host-side prep (layout/cast only): q/k transposed to [d,s] and packed fp8
(e4m3) in DoubleRow k-pair layout, v transposed bf16, per-head-pair Q/K
weights packed fp8 with a x16 gain (compensated in the exp scale), Wv/Wo
packed bf16.

Per-core algorithm (S=1024, D=1024, H=16 heads, E=64):
  - Q/K projections per head pair m as fp8 DoubleRow matmuls (K=256 per
    instruction); m=0/m=1 run first so the PE starts ~2 DMAs in; the
    projection for m+2 is emitted inside m's attention loop.
  - V-projection (bf16) produces V1[t, h, e|1] with a trailing ones
    column per head (softmax denominators fall out of the PV matmul);
    it runs inside m=0's loop after 4 score-only steps so the PE covers
    the vT/wv input transfers.
  - attention per (m, j): scoresT[t,s] = KT_slice.T @ QT (K=64); exp
    with all scaling folded in, split between ScalarE (spline exp) and
    DVE (bf16-bit-trick exp writing int16 exponent bits; its common-mode
    bias cancels exactly in softmax).  The hh=1 score halves live in two
    1-bank PSUM tiles so ACT and DVE never share a PSUM tile.  PV lags
    scores by two steps so the exp latency never stalls the PE.
  - per-m normalization (reciprocal + DRAM-broadcast + DVE multiply)
    runs inside the next m's loop; the tail-exposed m=7 uses a fast path
    (PE outer-product broadcast, no DRAM round trip) and FC prefix
    partials keep the PE busy during that chain.
  - FC: out = attT.T @ WoT + bo, PSUM ring 2 + 4-deep output ring.
"""

import numpy as np
from contextlib import ExitStack

import concourse.bass as bass
import concourse.mybir as mybir
import concourse.tile as tile
from concourse.bass_utils import run_bass_kernel_spmd

P = 128
S = 1024          # sequence length
DK = 1024         # qkv input dim
H = 16            # heads
E = 64            # per-head dim
HE = H * E        # 1024
OUT = 1024        # output dim
NT = S // P       # 8 s/t tiles
NK = DK // P      # 8 contraction tiles
NM = H // 2       # 8 head pairs
F32 = mybir.dt.float32
F32R = mybir.dt.float32r
BF16 = mybir.dt.bfloat16
I16 = mybir.dt.int16
FP8 = mybir.dt.float8e4
AF = mybir.ActivationFunctionType
ALU = mybir.AluOpType
SCALE = 1.0 / 32.0  # 1/sqrt(DK)

# ---- tuning knobs -------------------------------------------------------
EXP_DVE_COLS = 256      # columns per (m,j,hh) exp tile handled by DVE
USE_PROJ_DR = True      # fp8 DoubleRow for Q/K projections
USE_SCORES_DR = False   # fp8 DoubleRow for score matmuls

# Q/K weights are pre-scaled by WQK_GAIN on the host before fp8 rounding
# (pushes values out of the fp8 subnormal range); the resulting x256 gain
# on the scores is folded into the exp scale.
WQK_GAIN = 16.0 if USE_PROJ_DR else 1.0
ESCALE = SCALE / (WQK_GAIN * WQK_GAIN)

# bf16-bit-trick exp on DVE: bits16 = round(x * EXPA + EXPB) reinterpreted
# as bf16 gives approx exp(x * ESCALE).  The additive constant only shifts a
# common-mode factor which softmax normalization cancels exactly.
LOG2E = 1.4426950408889634
EXPA = ESCALE * LOG2E * 128.0
EXPB = 16256.0 - 4.75
MMPM = mybir.MatmulPerfMode


def _legalize_matmul_waits(nc):
    """This walrus build allows only ONE sync-wait command per Matmult.
    Move all but the last wait of any multi-wait matmul onto freshly
    inserted PE nops immediately before it — same engine queue, so the
    blocking semantics are identical."""
    SKIP = ("NoOp", "Br", "Halt", "Sem", "Event")
    k = 0
    for f in nc.m.functions:
        for b in f.blocks:
            out = []
            for inst in b.instructions:
                si = getattr(inst, "sync_info", None)
                tname = type(inst).__name__
                if (not any(s in tname for s in SKIP) and si is not None
                        and si.on_wait and len(si.on_wait) > 1):
                    waits = list(si.on_wait)
                    for w in waits[:-1]:
                        nop = mybir.InstNoOp(
                            name=f"legalize-nop-{k}", ins=[], outs=[])
                        k += 1
                        nop.engine = inst.engine
                        nop.sync_info = mybir.SyncInfo(
                            on_wait=[w], on_update=[])
                        out.append(nop)
                    inst.sync_info = mybir.SyncInfo(
                        on_wait=[waits[-1]], on_update=list(si.on_update))
                out.append(inst)
            b.instructions[:] = out
    return k


def build(legalize=True):
    nc = bass.Bass()
    QKDT = FP8 if USE_PROJ_DR else BF16
    # qT/kT layout: [ki, jj, t, s] with d = jj*256 + t*128 + ki
    qT_d = nc.dram_tensor("qT", (P, NK // 2, 2, S), QKDT, kind="ExternalInput")
    kT_d = nc.dram_tensor("kT", (P, NK // 2, 2, S), QKDT, kind="ExternalInput")
    vT_d = nc.dram_tensor("vT", (DK, S), BF16, kind="ExternalInput")
    # per head pair m: [ki, jj, t, he_pair]
    wq_d = nc.dram_tensor("wqp", (NM, P, NK // 2, 2, P), QKDT,
                          kind="ExternalInput")
    wk_d = nc.dram_tensor("wkp", (NM, P, NK // 2, 2, P), QKDT,
                          kind="ExternalInput")
    # [ko, ki, h*e]
    wv_d = nc.dram_tensor("wvp", (NK, P, HE), BF16, kind="ExternalInput")
    # Wo.T as [m, he_block, out]
    wo_d = nc.dram_tensor("woT", (NM, P, OUT), BF16, kind="ExternalInput")
    bo_d = nc.dram_tensor("bo", (OUT,), F32, kind="ExternalInput")
    out_d = nc.dram_tensor("out", (S, OUT), F32, kind="ExternalOutput")
    recip_d = nc.dram_tensor("recip_scratch", (H, S), BF16, kind="Internal")

    with tile.TileContext(nc) as tc, ExitStack() as ctx:
        const = ctx.enter_context(tc.tile_pool(name="const", bufs=1))
        xqk = ctx.enter_context(tc.tile_pool(name="xqk", bufs=1))
        v1p = ctx.enter_context(tc.tile_pool(name="v1p", bufs=NT))
        woTp = ctx.enter_context(tc.tile_pool(name="woTp", bufs=NM))
        wslp = ctx.enter_context(tc.tile_pool(name="wslp", bufs=4))
        qkp = ctx.enter_context(tc.tile_pool(name="qkp", bufs=6))
        ptp = ctx.enter_context(tc.tile_pool(name="ptp", bufs=6))
        sumsp = ctx.enter_context(tc.tile_pool(name="sumsp", bufs=1))
        rbcp = ctx.enter_context(tc.tile_pool(name="rbcp", bufs=2))
        outp = ctx.enter_context(tc.tile_pool(name="outp", bufs=4))
        # PSUM budget (8 banks): ps 'ps' ring1 x2 banks (hh0 scores + qtm
        # proj), 'psa'/'psb' ring1 x1 bank each (hh1 score halves: ACT-only
        # and DVE-only readers -> no cross-engine tile serialization),
        # ps_att 2x2 banks (attT accumulators / FC)
        ps = ctx.enter_context(tc.tile_pool(name="ps", bufs=1, space="PSUM"))
        ps_att = ctx.enter_context(
            tc.tile_pool(name="ps_att", bufs=2, space="PSUM"))
        ph1 = ExitStack()
        vwp = ph1.enter_context(tc.tile_pool(name="vwp", bufs=1))

        ones_h = const.tile([P, H], BF16, name="ones_h")
        nc.gpsimd.memset(ones_h[:], 1.0)
        ones_r = const.tile([1, E], BF16, name="ones_r")
        nc.gpsimd.memset(ones_r[:], 1.0)
        recbf_l = const.tile([1, 2, S], BF16, name="recbf_l")
        sums_m = [sumsp.tile([2, S], BF16, name=f"sums_m{i}")
                  for i in range(NM)]
        recip_m = [sumsp.tile([2, S], F32, name=f"recip_m{i}", tag="rcp",
                              bufs=2) for i in range(NM)]
        recbf_m = [sumsp.tile([2, S], BF16, name=f"recbf_m{i}", tag="rbf",
                              bufs=2) for i in range(NM)]
        # att65/attm pools are created after the vT/wv pool closes (their
        # first writes happen after vproj) so the allocations overlay
        late = {}

        # ---- input DMAs.  SP queue: wq/wk then qT (+rbc later); Pool
        # queue: kT, vT, wv, woT, gathers, out stores.  proj m=0 starts
        # as soon as wq0 + qT0 land.
        def load_wqk(m, queue=None):
            q_ = queue or nc.sync
            wq = wslp.tile([P, NK // 2, 2, P], QKDT, name=f"wq{m}", tag="wsl")
            wk = wslp.tile([P, NK // 2, 2, P], QKDT, name=f"wk{m}", tag="wsl")
            q_.dma_start(wq[:], wq_d[m])
            q_.dma_start(wk[:], wk_d[m])
            return wq, wk

        # single big tiles per tensor, loaded in chunks (fewer DMA fixed
        # overheads, paced for the m0 projection).  SP: wq0, qT chunks,
        # weights, vT; Pool: kT chunks, wv, bo, woT.
        QKDT = FP8 if USE_PROJ_DR else BF16
        qTt = xqk.tile([P, NK // 2, 2, S], QKDT, name="qTt", tag="qT")
        kTt = xqk.tile([P, NK // 2, 2, S], QKDT, name="kTt", tag="kT")
        wqk = [None, None]
        wqk[0] = load_wqk(0)
        for c in range(2):
            nc.sync.dma_start(qTt[:, c, 0, :], qT_d[:, c, 0, :])
            nc.sync.dma_start(qTt[:, c, 1, :], qT_d[:, c, 1, :])
            nc.gpsimd.dma_start(kTt[:, c, :, :], kT_d[:, c, :, :])
        wqk[1] = load_wqk(1)
        for c in range(2, 4):
            nc.sync.dma_start(qTt[:, c, :, :], qT_d[:, c, :, :])
            nc.gpsimd.dma_start(kTt[:, c, :, :], kT_d[:, c, :, :])

        vTt = vwp.tile([P, NK, S], BF16, name="vTt", tag="vT")
        wvt = vwp.tile([P, NK, HE], BF16, name="wvt", tag="wv")
        vT_v = vT_d.rearrange("(j p) s -> p j s", p=P)
        wv_v = wv_d.rearrange("j p e -> p j e")
        for c in range(4):
            nc.gpsimd.dma_start(vTt[:, 2 * c:2 * c + 2, :],
                                vT_v[:, 2 * c:2 * c + 2, :])
            nc.sync.dma_start(wvt[:, 2 * c:2 * c + 2, :],
                              wv_v[:, 2 * c:2 * c + 2, :])
        vT_t = [vTt[:, j, :] for j in range(NK)]
        wv_t = [wvt[:, j, :] for j in range(NK)]

        bo_bc = const.tile([P, OUT], F32, name="bo_bc")
        nc.gpsimd.dma_start(bo_bc[:], bo_d[None, :].to_broadcast((P, OUT)))
        woT_t = []
        for m in range(NM):
            t = woTp.tile([P, OUT], BF16, name=f"woT{m}", tag="woT")
            nc.gpsimd.dma_start(t[:], wo_d[m])
            woT_t.append(t)

        def proj_mms(wm, xt, pst, c0, c1):
            for sh in range(c0 // 512, c1 // 512):
                o0 = sh * 512 - c0
                if USE_PROJ_DR:
                    for jj in range(NK // 2):
                        nc.tensor.matmul(
                            pst[:, o0:o0 + 512],
                            wm[:, jj, :, :],
                            xt[:, jj, :, sh * 512:(sh + 1) * 512],
                            start=(jj == 0),
                            stop=(jj == NK // 2 - 1),
                            perf_mode=MMPM.DoubleRow)
                else:
                    for j in range(NK):
                        nc.tensor.matmul(
                            pst[:, o0:o0 + 512],
                            wm[:, j // 2, j % 2, :],
                            xt[:, j // 2, j % 2,
                               sh * 512:(sh + 1) * 512],
                            start=(j == 0), stop=(j == NK - 1))

        def proj_qk(m):
            """QT_m/KT_m [he_pair=128, s]; evacuated as bf16 per 512-col
            half so dependent scores can start on subtile waits."""
            wqm, wkm = wqk[m % 2]
            qkm = []
            for wi, (wm, xt, nm) in enumerate(
                    ((wqm, qTt, "qtm"), (wkm, kTt, "ktm"))):
                if wi == 0:
                    psts = [(ps.tile([P, S], F32, tag="ps",
                                     name=f"{nm}ps{m}"), 0, S)]
                else:
                    psts = [(ps.tile([P, 512], F32, tag="psa",
                                     name=f"{nm}psa{m}"), 0, 512),
                            (ps.tile([P, 512], F32, tag="psb",
                                     name=f"{nm}psb{m}"), 512, S)]
                t = qkp.tile([P, S], BF16, tag="qt", name=f"{nm}{m}")
                for pi, (pst, c0, c1) in enumerate(psts):
                    proj_mms(wm, xt, pst, c0, c1)
                    # evacuate immediately; balance the copies across
                    # DVE and ACT
                    ev = nc.scalar.copy if (wi == 0 or pi == 0) else \
                        nc.vector.tensor_copy
                    ev(t[:, c0:c1], pst[:, 0:c1 - c0])
                qkm.append(t)
            if m + 2 < NM:
                wqk[m % 2] = load_wqk(m + 2)
            return qkm

        # ---- phase A: Q/K proj for m=0,1 (starts the PE early; m=1 covers
        # the tail of the vT/wv input stream)
        qtm_next = proj_qk(0)
        qtm_next2 = proj_qk(1)

        # ---- phase B: V projection (+ ones column); alternating 1-bank
        # accumulators so the v1 evacuation overlaps the next half's matmuls
        v1_t = []

        def vproj_tile(i):
            v1 = v1p.tile([P, H, E + 1], BF16, tag="v1", name=f"v1_{i}")
            nc.vector.tensor_copy(v1[:, :, E], ones_h[:])
            for nh, tg in ((0, "psa"), (1, "psb")):
                pst = ps.tile([P, 512], F32, tag=tg, name=f"vp{i}_{nh}")
                for j in range(NK):
                    nc.tensor.matmul(
                        pst[:],
                        vT_t[j][:, i * P:(i + 1) * P],
                        wv_t[j][:, nh * 512:(nh + 1) * 512],
                        start=(j == 0), stop=(j == NK - 1))
                nc.vector.tensor_copy(
                    v1[:, nh * 8:(nh + 1) * 8, 0:E],
                    pst[:].rearrange("p (h e) -> p h e", e=E))
            v1_t.append(v1)


        # ---- phase C: attention m loop
        attm_t = {}

        def normalize_m(m):
            """per-m: reciprocal of the pair's denominators, DRAM-broadcast,
            normalize att65 -> attm on DVE.  Runs inside the next m's loop
            so only m=7's chain is tail-exposed."""
            nc.vector.reciprocal(recip_m[m][:], sums_m[m][:])
            nc.vector.tensor_copy(recbf_m[m][:], recip_m[m][:])
            nc.sync.dma_start(recip_d[2 * m:2 * m + 2, :], recbf_m[m][:])
            attm = late['attp'].tile([P, S], BF16, tag="attm",
                                     name=f"attm{m}")
            attm_t[m] = attm
            for hh in range(2):
                rbc = rbcp.tile([E, S], BF16, tag="rbc",
                                name=f"rbc{m}_{hh}")
                nc.sync.dma_start(
                    rbc[:], recip_d[2 * m + hh][None, :].to_broadcast((E, S)))
                nc.vector.tensor_tensor(
                    attm[hh * E:(hh + 1) * E, :],
                    late['att65'][0:E, 2 * m + hh, :], rbc[:], ALU.mult)

        att_prev = {}
        fc_pre = {}
        pend = []
        pops = [0]

        def pv_step(att_t, mm_, j, hh, pparts):
            for sh in range(2):
                nc.tensor.matmul(
                    att_t[hh][:, sh * 512:(sh + 1) * 512],
                    v1_t[j][:, 2 * mm_ + hh, :],
                    pparts[sh],
                    start=(j == 0), stop=(j == NT - 1))

        def evac_att(pm):
            # attendedT + denominator row of pair pm (unnormalized, bf16),
            # deferred past the next pair's first exps.  DVE mid-loop (ACT
            # is the boundary bottleneck); ACT for the tail-exposed m7.
            eng = nc.scalar.copy if pm == NM - 1 else nc.vector.tensor_copy
            for hh in range(2):
                eng(late['att65'][:, 2 * pm + hh, :], att_prev[pm][hh][:])
                if pm < NM - 1:
                    nc.gpsimd.dma_start(
                        sums_m[pm][hh:hh + 1, :],
                        late['att65'][E:E + 1, 2 * pm + hh, :])

        for m in range(NM):
            qtm, ktm = qtm_next
            if m + 1 < NM:
                qtm_next = qtm_next2

            att_t = {}
            for hh in range(2):
                att_t[hh] = ps_att.tile([E + 1, S], F32, tag="attps",
                                        name=f"att{m}_{hh}")

            # software-pipelined: PV lags scores by TWO steps (four during
            # m0's input-transfer window), so the ~1.4us exp chain latency
            # never stalls the PE.  The backlog carries across m boundaries
            # so the pipeline never refills from empty.
            LAG = 4 if m == 0 else 2
            for j in range(NT):
                # m0: four score-only steps run first, then the V
                # projection (fills the input-transfer window with PE work)
                if m == 0 and j == LAG - 1:
                    for i in range(NT):
                        vproj_tile(i)
                    ph1.close()
                    late["att65p"] = ctx.enter_context(
                        tc.tile_pool(name="att65p", bufs=1))
                    late["attp"] = ctx.enter_context(
                        tc.tile_pool(name="attp", bufs=NM))
                    late["att65"] = late["att65p"].tile(
                        [E + 1, H, S], BF16, name="att65")
                if j == 3 and m >= 1:
                    evac_att(m - 1)
                # previous pair's normalization
                if j == 4 and m >= 1:
                    normalize_m(m - 1)
                cur = []
                for hh in range(2):
                    if len(pend) > LAG or (len(pend) == LAG and hh == 0):
                        if pops[0] < 2:
                            pv_step(*pend[0][pops[0]])
                            pops[0] += 1
                        if pops[0] == 2:
                            pend.pop(0)
                            pops[0] = 0
                    hs = slice(hh * E, (hh + 1) * E)
                    if hh == 1 and EXP_DVE_COLS > 0:
                        # hh=1: two 1-bank score tiles; ACT reads only sca,
                        # DVE reads only scb -> no cross-engine ordering
                        sca = ps.tile([P, 512], F32, tag="psa",
                                      name=f"sca{m}_{j}")
                        scb = ps.tile([P, 512], F32, tag="psb",
                                      name=f"scb{m}_{j}")
                        for sh, sct in ((0, sca), (1, scb)):
                            nc.tensor.matmul(
                                sct[:],
                                ktm[hs, j * P:(j + 1) * P],
                                qtm[hs, sh * 512:(sh + 1) * 512],
                                start=True, stop=True)
                        pa = ptp.tile([P, 512], BF16, tag="pta",
                                      name=f"pa{m}_{j}", bufs=5)
                        pb = ptp.tile([P, 512], I16, tag="ptb",
                                      name=f"pb{m}_{j}", bufs=5)
                        nc.scalar.activation(pa[:], sca[:],
                                             AF.Exp, scale=ESCALE)
                        nc.vector.tensor_scalar(
                            pb[:], scb[:],
                            EXPA, EXPB, ALU.mult, ALU.add)
                        cur.append((att_t, m, j, hh,
                                    (pa[:], pb.bitcast(BF16)[:])))
                    else:
                        sc = ps.tile([P, S], F32, tag="ps",
                                     name=f"sc{m}_{j}_{hh}")
                        for sh in range(2):
                            nc.tensor.matmul(
                                sc[:, sh * 512:(sh + 1) * 512],
                                ktm[hs, j * P:(j + 1) * P],
                                qtm[hs, sh * 512:(sh + 1) * 512],
                                start=True, stop=True)
                        ptile = ptp.tile([P, S], BF16, tag="pt",
                                         name=f"p{m}_{j}_{hh}", bufs=5)
                        nc.scalar.activation(ptile[:], sc[:],
                                             AF.Exp, scale=ESCALE)
                        cur.append((att_t, m, j, hh, (ptile[:, 0:512],
                                                      ptile[:, 512:S])))
                # Q/K proj of m+2 after the last scores: PE fills the final
                # exp latencies; its evacuations overlap the PV drain
                if j == NT - 1 and m + 2 < NM:
                    qtm_next2 = proj_qk(m + 2)
                pend.append(cur)
            # drain down to the steady two-step backlog (fully at the end)
            keep = 0 if m == NM - 1 else 2
            while len(pend) > keep or (len(pend) == keep and pops[0] > 0):
                pv_step(*pend[0][pops[0]])
                pops[0] += 1
                if pops[0] == 2:
                    pend.pop(0)
                    pops[0] = 0
                if len(pend) == keep and pops[0] == 0:
                    break

            att_prev[m] = att_t
            if m == NM - 1:
                # FC prefix partials (m0..m5) fill the PE while the final
                # normalize chain runs; completed in phase D
                for oh, (pool_, tg) in ((0, (ps, "ps")), (1, (ps, "psa"))):
                    pso = pool_.tile([P, 512], F32, tag=tg,
                                     name=f"fcpre{oh}")
                    for mm_ in range(NM - 2):
                        nc.tensor.matmul(
                            pso[:],
                            attm_t[mm_][:, 0:P],
                            woT_t[mm_][:, oh * 512:(oh + 1) * 512],
                            start=(mm_ == 0), stop=False)
                    fc_pre[oh] = pso
                evac_att(m)

        # m=7 tail-exposed normalize: reciprocal straight off att65, PE
        # outer-product broadcast (no DRAM round trip), DVE mult from PSUM
        m7 = NM - 1
        attm = late['attp'].tile([P, S], BF16, tag="attm",
                                 name=f"attm{m7}")
        attm_t[m7] = attm
        for hh in range(2):
            with nc.allow_low_precision(reason="bf16 reciprocal matches "
                                        "the recip_bf path precision"):
                nc.vector.reciprocal(
                    recbf_l[:, hh, :],
                    late['att65'][E:E + 1, 2 * m7 + hh, :])
            for sh in range(2):
                rps = ps.tile([E, 512], F32, tag="psb",
                              name=f"rps{hh}_{sh}")
                nc.tensor.matmul(
                    rps[:],
                    ones_r[:],
                    recbf_l[:, hh, sh * 512:(sh + 1) * 512],
                    start=True, stop=True)
                nc.vector.tensor_tensor(
                    attm[hh * E:(hh + 1) * E, sh * 512:(sh + 1) * 512],
                    late['att65'][0:E, 2 * m7 + hh, sh * 512:(sh + 1) * 512],
                    rps[:], ALU.mult)

        # ---- phase D: FC
        for st in range(NT):
            for oh in range(2):
                if st == 0 and oh in fc_pre:
                    pso = fc_pre[oh]
                    m_range = range(NM - 2, NM)
                else:
                    pso = ps_att.tile([P, 512], F32, tag="attps",
                                      name=f"fc{st}_{oh}")
                    m_range = range(NM)
                for m in m_range:
                    nc.tensor.matmul(
                        pso[:],
                        attm_t[m][:, st * P:(st + 1) * P],
                        woT_t[m][:, oh * 512:(oh + 1) * 512],
                        start=(m == 0), stop=(m == NM - 1))
                ot = outp.tile([P, 512], F32, tag="out", name=f"out{st}_{oh}")
                nc.vector.tensor_tensor(
                    ot[:], pso[:], bo_bc[:, oh * 512:(oh + 1) * 512],
                    ALU.add)
                (nc.sync if (st + oh) % 2 == 0 else nc.gpsimd).dma_start(
                    out_d[st * P:(st + 1) * P, oh * 512:(oh + 1) * 512], ot[:])
    if legalize:
        _legalize_matmul_waits(nc)
    return nc


_NC_CACHE = {}


def _get_nc():
    if "nc" not in _NC_CACHE:
        _NC_CACHE["nc"] = build()
    return _NC_CACHE["nc"]


def _host_pack(query, key, value, Wq, Wk, Wv, Wo, bo):
    """Per-problem host-side layout prep (transpose + cast only)."""
    bf16 = mybir.dt.np(BF16)
    qkdt = mybir.dt.np(FP8) if USE_PROJ_DR else bf16
    # q/k: [s, d] -> [ki, jj, t, s] with d = jj*256 + t*128 + ki
    def packx(x):
        t = x.transpose(0, 2, 1).reshape(-1, NK // 2, 2, P, S)
        return np.ascontiguousarray(t.transpose(0, 3, 1, 2, 4)).astype(qkdt)

    qT = packx(query)
    kT = packx(key)
    vT = np.ascontiguousarray(value.transpose(0, 2, 1)).astype(bf16)

    # Wq [h, d, e] with d = jj*256 + t*128 + ki, h = 2m + hh ->
    # [m, ki, jj, t, (hh e)]
    def packw(W):
        t = (W * WQK_GAIN).reshape(NM, 2, NK // 2, 2, P, E)
        t = t.transpose(0, 4, 2, 3, 1, 5)
        return np.ascontiguousarray(
            t.reshape(NM, P, NK // 2, 2, P)).astype(qkdt)

    wqp = packw(Wq)
    wkp = packw(Wk)
    # Wv [h, d, e] -> [ko, ki, (h e)]
    wvp = np.ascontiguousarray(
        Wv.reshape(H, NK, P, E).transpose(1, 2, 0, 3).reshape(NK, P, HE)
    ).astype(bf16)
    # Wo [out, he] -> [m, he_block, out]
    woT = np.ascontiguousarray(
        Wo.T.reshape(NM, P, OUT)).astype(bf16)
    return qT, kT, vT, wqp, wkp, wvp, woT, bo.astype(np.float32)


def kernel(query, key, value, Wq, Wk, Wv, Wo, bo, **run_kwargs):
    query = np.asarray(query, dtype=np.float32)
    key = np.asarray(key, dtype=np.float32)
    value = np.asarray(value, dtype=np.float32)
    Wq = np.asarray(Wq, dtype=np.float32)
    Wk = np.asarray(Wk, dtype=np.float32)
    Wv = np.asarray(Wv, dtype=np.float32)
    Wo = np.asarray(Wo, dtype=np.float32)
    bo = np.asarray(bo, dtype=np.float32)
    B = query.shape[0]
    assert B == 8, f"expected batch 8, got {B}"

    qT, kT, vT, wqp, wkp, wvp, woT, bo32 = _host_pack(
        query, key, value, Wq, Wk, Wv, Wo, bo)

    nc = _get_nc()
    in_maps = []
    for b in range(B):
        in_maps.append({
            "qT": qT[b], "kT": kT[b], "vT": vT[b],
            "wqp": wqp, "wkp": wkp, "wvp": wvp, "woT": woT, "bo": bo32,
        })
    res = run_bass_kernel_spmd(nc, in_maps, core_ids=list(range(B)),
                               **run_kwargs)
    out = np.stack([r["out"] for r in res.results], axis=0)
    if run_kwargs.get("trace"):
        _NC_CACHE["last_result"] = res
    return out
